# revision 22
# baseline (speedup 1.0000x reference)
"""Trainium2 Bass kernel for the nn_Detect head (3-level YOLO-style decode).

Strategy: data-parallel over batch (8 images -> 8 NeuronCores). Per core:
  - 3x3 convs via 1D Winograd F(2,3) ALONG Y (direct 3-tap slide along x):
    host precomputes V = B^T-transformed input row-quads (4 pos planes over
    H/2 row-pairs), device runs 12 (pos,kx) shifted matmuls per k-pair in
    fp8 e4m3 + DoubleRow accumulating 4 PSUM pos-banks, then a 4-pass
    vector/gpsimd recombine (o_even = m0+m1+m2, o_odd = m1-m2-m3) writes
    interleaved row-pairs straight into the bf16 act tiles (full-row
    bursts, so no stride-2 AP poison). 6/9 the MACs of direct conv; the
    direct version measured at 100% of the DoubleRow PE roofline
    (~70 TMAC/s), so fewer MACs is the only lever.
  - fp8 weights are host-prescaled by WSCALE=64 (e4m3 subnormal range);
    the 1/WSCALE now folds into the 1x1 HEAD weights (the old rescale rode
    the PSUM->SBUF copy, which the recombine replaced).
  - The wh head channels (decode multiplies conv noise by anchors up to
    373 at level 2) are recomputed exactly via a host-folded
    (w2b_wh . w2a) [6,1024] fp16 Winograd-y conv straight from a fp16 V;
    the result is PE-transposed into [px, 6] and swapped into decode's wh
    source.
  - 1x1 head convs computed as act_chunk.T @ W so the matmul itself emits
    [px, ch] tiles (transpose for free), 128 pixels on partitions.
  - decode (sigmoid/grid/anchor/dims) as wide-partition vector/scalar ops
    writing straight into per-(level, anchor) output staging tiles;
    orientation L2-normalize deferred to one Sqrt + DVE-reciprocal pass
    per level; stores deferred into the level-2 compute window, issued
    from three engines (gpsimd/scalar/sync).
  - startup: first conv's weight tile preloaded on the idle gpsimd queue,
    level-0 V row-split across scalar/sync queues ahead of the constants.
Host side packs/pads/transposes all inputs; bias adds are emitted only when
the bias tensors are nonzero (the spec fills them with zeros).
Weight prefetch is 5-deep (l0/l1) / 4-deep (l2), with l1's stream on the
GPSIMD queue and l2's on sync (hardware-dynamic DMA triggers are engine
instructions; see git history of the direct-conv variant for rationale).
Direct-conv baseline measured 355us; this build targets ~270-285us.
"""

import numpy as np
import ml_dtypes

bf16 = ml_dtypes.bfloat16

BS = 8
NCORES = 8
NO2D = 8
NO3D = 9
NOUT = 17
NROWS = 25200

# (C, H, W, stride, ty_blocks)  -- ty_blocks partition H/2 row-pairs
LEVELS = [
    (256, 80, 80, 8.0, [6, 6, 6, 6, 6, 6, 4]),
    (512, 40, 40, 16.0, [10, 10]),
    (1024, 20, 20, 32.0, [10]),
]
ANCHORS = np.array(
    [
        [[10, 13], [16, 30], [33, 23]],
        [[30, 61], [62, 45], [59, 119]],
        [[116, 90], [156, 198], [373, 326]],
    ],
    np.float32,
)

_S = [H * W for (_, H, W, _, _) in LEVELS]              # 6400, 1600, 400
_NCH = [(s + 127) // 128 for s in _S]                   # 50, 13, 4
_ROW0 = [0, 3 * _S[0], 3 * _S[0] + 3 * _S[1]]           # level row offsets
_NBOFF = []
_off = 0
for _l in range(3):
    for _a in range(3):
        _NBOFF.append(_off)
        _off += _NCH[_l]
_NBTOT = _off                                           # 201

FP8B = ((True, True), (True, True), (True, True))
WSCALE = 64.0        # host premultiplier for fp8 weights (std 0.01 is subnormal)
_W2P = [88, 48, 24]  # x-padded width per level: 4*(H/2)*W2P % 16 == 0 for k-pair APs

# F(2,3) transform matrices (host side)
_G = np.array([[1, 0, 0], [0.5, 0.5, 0.5], [0.5, -0.5, 0.5], [0, 0, 1]], np.float32)
_BT = np.array(
    [[1, 0, -1, 0], [0, 1, 1, 0], [0, -1, 1, 0], [0, 1, 0, -1]], np.float32
)

_PROGRAM_CACHE = {}


def _groups(nchunks, g=16):
    out = []
    k0 = 0
    while k0 < nchunks:
        gn = min(g, nchunks - k0)
        out.append((k0, gn))
        k0 += gn
    return out


def _build_program(bias_flags):
    import concourse.mybir as mybir
    import concourse.tile as tile
    from concourse import bacc

    (has_ba, has_bb) = bias_flags  # tuples of 6 bools: (l, branch) order

    import concourse.masks as masks

    nc = bacc.Bacc(None)
    f32 = mybir.dt.float32
    b16 = mybir.dt.float16
    cdtb = [[mybir.dt.float8e4 if f8 else b16 for f8 in lv] for lv in FP8B]
    DR = mybir.MatmulPerfMode.DoubleRow

    fps = []
    was = []
    wbs = []
    grids = []
    for l, (C, H, W, _, _) in enumerate(LEVELS):
        Q = C // 128
        Hh = H // 2
        W2 = _W2P[l]
        p0 = nc.declare_dram_parameter(f"v{l}", [128, Q, 4, Hh, W2], cdtb[l][0], isOutput=False)
        fps.append([p0, p0])
        was.append(
            [
                nc.declare_dram_parameter(
                    f"wa{b}{l}", [Q, 128, Q, 12, 128], cdtb[l][bi], isOutput=False
                )
                for bi, b in enumerate((2, 3))
            ]
        )
        wbs.append(
            [
                nc.declare_dram_parameter(
                    f"wb{b}{l}", [128, Q, NO2D * 3 if b == 2 else NO3D * 3], b16, isOutput=False
                )
                for b in (2, 3)
            ]
        )
        grids.append(
            nc.declare_dram_parameter(f"grid{l}", [128, _NCH[l], 2], f32, isOutput=False)
        )
    anch = nc.declare_dram_parameter("anch", [128, 3, 3, 2], f32, isOutput=False)
    # folded wh path for level 2: exact fp16 winograd-y conv for the 6 wh channels
    Q2 = LEVELS[2][0] // 128
    v2h = nc.declare_dram_parameter(
        "v2h", [128, Q2, 4, LEVELS[2][1] // 2, _W2P[2]], b16, isOutput=False
    )
    wfoldp = nc.declare_dram_parameter("wfold", [128, Q2, 12, 6], b16, isOutput=False)
    has_foldb = bias_flags[0][4] or bias_flags[1][4]
    if has_foldb:
        foldbp = nc.declare_dram_parameter("foldb", [6, 1], f32, isOutput=False)
    bas = {}
    bbs = {}
    for l, (C, H, W, _, _) in enumerate(LEVELS):
        Q = C // 128
        for bi, b in enumerate((2, 3)):
            if has_ba[l * 2 + bi]:
                bas[(l, b)] = nc.declare_dram_parameter(f"ba{b}{l}", [128, Q], f32, isOutput=False)
            if has_bb[l * 2 + bi]:
                nchn = NO2D * 3 if b == 2 else NO3D * 3
                bbs[(l, b)] = nc.declare_dram_parameter(f"bb{b}{l}", [128, nchn], f32, isOutput=False)
    out = nc.declare_dram_parameter("out", [NROWS, NOUT], f32, isOutput=True)

    with tile.TileContext(nc) as tc:
        from contextlib import ExitStack

        with ExitStack() as ctx:
            cpool = ctx.enter_context(tc.tile_pool(name="consts", bufs=1))
            spool = ctx.enter_context(tc.tile_pool(name="stage", bufs=1))
            ipool = ctx.enter_context(tc.tile_pool(name="inbuf", bufs=1))
            a2pool = ctx.enter_context(tc.tile_pool(name="act2", bufs=1))
            a3pool = ctx.enter_context(tc.tile_pool(name="act3", bufs=1))
            wpool = ctx.enter_context(tc.tile_pool(name="w3", bufs=2))
            p3pool = ctx.enter_context(tc.tile_pool(name="psum3", bufs=5, space="PSUM"))
            hpool = ctx.enter_context(tc.tile_pool(name="hp", bufs=3, space="PSUM"))
            scpool = ctx.enter_context(tc.tile_pool(name="scratch", bufs=2))

            # ---- critical-path startup DMAs (before consts in queue order):
            # the very first conv matmul needs l0's j0 weight tile and V
            # ty-rows 0:6 of both chunks; give each its own queue head.
            C0, H0, _, _, tyb0 = LEVELS[0]
            Q0 = C0 // 128
            Hh0 = H0 // 2
            it0_l0 = ipool.tile(
                [128, Q0, 4, Hh0, _W2P[0]], cdtb[0][0], tag="inb0", name="inb0"
            )
            wt00 = wpool.tile([128, Q0, 12, 128], cdtb[0][0], tag="w3a", bufs=4)
            # slice-granular deps: the first matmuls touch pos plane 1 (taps
            # 3:9) -- land those bytes first so the PE starts ~3us earlier
            nc.gpsimd.dma_start(wt00[:, :, 3:9], was[0][0][0, :, :, 3:9])
            nc.scalar.dma_start(it0_l0[:, 0, 1:3, 0:6], fps[0][0][:, 0, 1:3, 0:6])
            nc.sync.dma_start(it0_l0[:, 1, 1:3, 0:6], fps[0][0][:, 1, 1:3, 0:6])
            nc.scalar.dma_start(it0_l0[:, 0, 0, 0:6], fps[0][0][:, 0, 0, 0:6])
            nc.sync.dma_start(it0_l0[:, 1, 0, 0:6], fps[0][0][:, 1, 0, 0:6])
            nc.scalar.dma_start(it0_l0[:, 0, 3, 0:6], fps[0][0][:, 0, 3, 0:6])
            nc.sync.dma_start(it0_l0[:, 1, 3, 0:6], fps[0][0][:, 1, 3, 0:6])
            nc.gpsimd.dma_start(wt00[:, :, 0:3], was[0][0][0, :, :, 0:3])
            nc.gpsimd.dma_start(wt00[:, :, 9:12], was[0][0][0, :, :, 9:12])
            # V l0 is 2x the bytes of the old raw-input layout; stream the
            # rest 3-queue-parallel ahead of the constants so j0's last
            # blocks (~27us in) don't wait on it
            nc.scalar.dma_start(it0_l0[:, 0, :, 6:23], fps[0][0][:, 0, :, 6:23])
            nc.gpsimd.dma_start(it0_l0[:, 1, :, 6:23], fps[0][0][:, 1, :, 6:23])
            nc.sync.dma_start(it0_l0[:, 0, :, 23:40], fps[0][0][:, 0, :, 23:40])
            nc.scalar.dma_start(it0_l0[:, 1, :, 23:40], fps[0][0][:, 1, :, 23:40])
            # prefetch l1's first weight tile + V too: their natural triggers
            # sit behind all of l0's compute and fire ~4us too late
            wt10 = wpool.tile([128, 4, 12, 128], cdtb[1][0], tag="w3a", bufs=4)
            nc.gpsimd.dma_start(wt10[:], was[1][0][0])
            C1, H1, _, _, _ = LEVELS[1]
            it0_l1 = ipool.tile(
                [128, 4, 4, H1 // 2, _W2P[1]], cdtb[1][0], tag="inb1", name="inb1"
            )
            for q in range(4):
                (nc.scalar if q % 2 == 0 else nc.gpsimd).dma_start(
                    it0_l1[:, q], fps[1][0][:, q]
                )

            # ---- constants ----
            gts = []
            wbt = []
            for l, (C, H, W, _, _) in enumerate(LEVELS):
                Q = C // 128
                gt = cpool.tile([128, _NCH[l], 2], f32, tag=f"grid{l}")
                nc.sync.dma_start(gt[:], grids[l][:])
                gts.append(gt)
                w2t = cpool.tile([128, Q, NO2D * 3], b16, tag=f"wb2{l}")
                nc.sync.dma_start(w2t[:], wbs[l][0][:])
                w3t = cpool.tile([128, Q, NO3D * 3], b16, tag=f"wb3{l}")
                nc.sync.dma_start(w3t[:], wbs[l][1][:])
                wbt.append((w2t, w3t))
            ancht = cpool.tile([128, 3, 3, 2], f32)
            nc.sync.dma_start(ancht[:], anch[:])
            epst = cpool.tile([128, 1], f32)
            nc.vector.memset(epst[:], 1e-24)
            wft = cpool.tile([128, Q2, 12, 6], b16, tag="wfold")
            nc.scalar.dma_start(wft[:], wfoldp[:])
            ident = cpool.tile([8, 8], f32, tag="ident")
            masks.make_identity(nc, ident[:])
            if has_foldb:
                fbt = cpool.tile([6, 1], f32, tag="foldb")
                nc.scalar.dma_start(fbt[:], foldbp[:])
            bat = {}
            bbt = {}
            for (l, b), p in bas.items():
                t = cpool.tile(list(p.shape), f32, tag=f"ba{b}{l}")
                nc.sync.dma_start(t[:], p[:])
                bat[(l, b)] = t
            for (l, b), p in bbs.items():
                t = cpool.tile(list(p.shape), f32, tag=f"bb{b}{l}")
                nc.sync.dma_start(t[:], p[:])
                bbt[(l, b)] = t

            # ---- staging + norm buffers (persist to end) ----
            st = [
                [
                    spool.tile([128, _NCH[l], NOUT], f32, tag=f"st{l}{a}", name=f"st{l}{a}")
                    for a in range(3)
                ]
                for l in range(3)
            ]
            nb = spool.tile([128, _NBTOT, 2], f32)

            def _emit_norm(l):
                # orientation 1/sqrt(norm^2) for all three anchors of level l
                loff = _NBOFF[l * 3]
                nbl = nb[:, loff : loff + 3 * _NCH[l], :]
                nc.scalar.activation(nbl, nbl, mybir.ActivationFunctionType.Sqrt, bias=epst[:])
                nc.vector.reciprocal(nbl, nbl)
                for a in range(3):
                    noff = _NBOFF[l * 3 + a]
                    ori = st[l][a][:, :, 10:14].rearrange("p k (j t) -> p k j t", t=2)
                    rinv = nb[:, noff : noff + _NCH[l], :][:, :, :, None].to_broadcast(
                        [128, _NCH[l], 2, 2]
                    )
                    nc.vector.tensor_tensor(ori, ori, rinv, mybir.AluOpType.mult)

            def _emit_stores(l, S, c0=0, c1=None):
                kfull, rem = divmod(S, 128)
                if c1 is None:
                    c1 = _NCH[l]
                ce = min(c1, kfull)
                # one engine per anchor for the exposed level-2 stores at the
                # very end; l0/l1 stores all ride the near-idle sync engine so
                # their ~0.7us issue costs don't wedge the compute engines
                dengs = (nc.gpsimd, nc.scalar, nc.sync) if l == 2 else (nc.sync,) * 3
                for a in range(3):
                    row0 = _ROW0[l] + a * S
                    eng = dengs[a]
                    if ce > c0:
                        eng.dma_start(
                            out[row0 + c0 * 128 : row0 + ce * 128, :].rearrange(
                                "(k p) c -> p k c", p=128
                            ),
                            st[l][a][:, c0:ce, :],
                        )
                    if rem and c1 > kfull:
                        eng.dma_start(
                            out[row0 + kfull * 128 : row0 + S, :],
                            st[l][a][:rem, kfull, :],
                        )

            rc_ctr = 0
            pending_tails = []
            inb = [None, None, None]
            for l, (C, H, W, stride, tyblocks) in enumerate(LEVELS):
                Q = C // 128
                QP = Q // 2
                Hh = H // 2
                S = H * W

                # load this level's V planes; inputs ride the scalar/gpsimd
                # queues so they never queue behind weights (sync queue).
                # Level 0 additionally splits ty-rows so the first matmul's
                # dependency is small.
                W2 = _W2P[l]
                iengs = (nc.scalar, nc.gpsimd)
                ei = 0
                if l == 0:
                    # fully DMA'd pre-consts across all three queues
                    it0 = it0_l0
                elif l == 1:
                    it0 = it0_l1  # prefetched at startup
                    # hoist l2's V loads here: the scalar/gpsimd engines reach
                    # these triggers right after l0's compute (~140us), so the
                    # tensors land ~50us before l2 needs them instead of
                    # arriving mid-stall (the inb0 WAR on l0's conv reads
                    # resolves at ~135us; emitting them inside the l2 block
                    # fired them ~200us)
                    C2, H2, W2_, _, _ = LEVELS[2]
                    Q2l = C2 // 128
                    it16 = ipool.tile(
                        [128, Q2l, 4, H2 // 2, _W2P[2]], b16, tag="inb2c", name="inb2c"
                    )
                    for q in range(Q2l):
                        iengs[ei % 2].dma_start(it16[:, q], v2h[:, q])
                        ei += 1
                    it0_l2 = ipool.tile(
                        [128, Q2l, 4, H2 // 2, _W2P[2]], cdtb[2][0], tag="inb0", name="inb2"
                    )
                    for q in range(Q2l):
                        iengs[ei % 2].dma_start(it0_l2[:, q], fps[2][0][:, q])
                        ei += 1
                else:
                    it0 = it0_l2
                it1 = it0
                inb[l] = [it0, it1]

                fhT = None
                if l == 2:
                    # exact fp16 winograd-y conv for the 6 wh head channels,
                    # folded from w2b_wh . w2a on host -- immune to the fp8
                    # conv noise that the 373-max anchors would amplify.
                    fh = [
                        p3pool.tile([128, 512], f32, tag="blk", name="foldmm")
                        for _ in range(4)
                    ]
                    Nf = Hh * W
                    for p in range(4):
                        for q in range(Q):
                            for kx in range(3):
                                nc.tensor.matmul(
                                    fh[p][:6, :Nf],
                                    wft[:, q, p * 3 + kx, :],
                                    it16[:, q, p, :, kx : kx + W],
                                    start=(q == 0 and kx == 0),
                                    stop=(q == Q - 1 and kx == 2),
                                )
                    fhs = scpool.tile([6, 400], f32, tag="fhs")
                    fhc = scpool.tile([6, 256], f32, tag="fhc")
                    fhu = scpool.tile([6, 256], f32, tag="fhu")
                    add_ = mybir.AluOpType.add
                    sub_ = mybir.AluOpType.subtract
                    slf = fhs[:, :].rearrange("p (t e w) -> p t e w", e=2, w=W)
                    ufr = fhu[:, :Nf].rearrange("p (t w) -> p t w", w=W)
                    nc.scalar.copy(fhc[:, :Nf], fh[1][:6, :Nf])
                    nc.vector.tensor_tensor(fhu[:, :Nf], fhc[:, :Nf], fh[2][:6, :Nf], add_)
                    nc.vector.tensor_tensor(
                        slf[:, :, 0, :], ufr,
                        fh[0][:6, :Nf].rearrange("p (t w) -> p t w", w=W), add_,
                    )
                    nc.vector.tensor_tensor(fhu[:, :Nf], fhc[:, :Nf], fh[2][:6, :Nf], sub_)
                    nc.vector.tensor_tensor(
                        slf[:, :, 1, :], ufr,
                        fh[3][:6, :Nf].rearrange("p (t w) -> p t w", w=W), sub_,
                    )
                    if has_foldb:
                        nc.vector.tensor_scalar(
                            fhs[:], fhs[:], 1.0, fbt[:, 0:1],
                            mybir.AluOpType.mult, mybir.AluOpType.add,
                        )

                w2t, w3t = wbt[l]
                sig = mybir.ActivationFunctionType.Sigmoid
                mult = mybir.AluOpType.mult
                add = mybir.AluOpType.add

                if l == 2 and pending_tails:
                    # flush levels 0/1 fixup+stores now: their deps are long
                    # ready, level-2's weight DMAs already outrank them, and
                    # the remaining level-2 compute hides the flood
                    for fn in pending_tails:
                        fn()
                    pending_tails.clear()

                # the two conv branches SHARE one act buffer: conv-w2a fills
                # it, h2 heads+decode consume it, then conv-w3a overwrites it
                # (tile WAR deps order the recombine writes after the reads)
                border = ((1, 3), (0, 2)) if l == 2 else ((0, 2), (1, 3))
                for bi, b in border:
                    act = a2pool.tile([128, Q, S], b16, tag=f"act_{l % 2}", name=f"act{b}_{l}")
                    itb = inb[l][bi]
                    batile = bat.get((l, b))
                    for j in range(Q):
                        if l == 0 and bi == 0 and j == 0:
                            wt = wt00  # preloaded on the gpsimd queue pre-consts
                        elif l == 1 and bi == 0 and j == 0:
                            wt = wt10  # prefetched at startup
                        else:
                            wtag, wbufs = ("w3b", 3) if l == 2 else ("w3a", 4)
                            wt = wpool.tile([128, Q, 12, 128], cdtb[l][bi], tag=wtag, bufs=wbufs)
                            # l1 weights ride the gpsimd queue (its stream is
                            # empty early so triggers fire early); l2's 25MB
                            # splits across sync (b2) and scalar (b3) -- the
                            # 12-tap winograd stream needs ~300GB/s in the l2
                            # window, too much for one queue.
                            if l == 1:
                                weng = nc.gpsimd
                            elif l == 2 and bi == 1:
                                weng = nc.scalar
                            else:
                                weng = nc.sync
                            weng.dma_start(wt[:], was[l][bi][j])
                        t0 = 0
                        for tn in tyblocks:
                            N = tn * W
                            # alloc+emit pos in order (1,2,0,3): banks then
                            # free (via c1, c2, sl_even, sl_odd) in exactly
                            # the order the next block's groups claim them
                            mtmp = {
                                p: p3pool.tile([128, 512], f32, tag="blk", name="blk")
                                for p in (1, 2, 0, 3)
                            }
                            ms = [mtmp[p] for p in range(4)]
                            for p in (1, 2, 0, 3):
                                for qp in range(QP):
                                    for kx in range(3):
                                        nc.tensor.matmul(
                                            ms[p][:, :N],
                                            wt[:, 2 * qp : 2 * qp + 2, p * 3 + kx, :],
                                            itb[
                                                :, 2 * qp : 2 * qp + 2, p,
                                                t0 : t0 + tn, kx : kx + W,
                                            ],
                                            start=(qp == 0 and kx == 0),
                                            stop=(qp == QP - 1 and kx == 2),
                                            perf_mode=DR,
                                        )
                            # recombine: o_even = m0+m1+m2, o_odd = m1-m2-m3.
                            # Engines read at most ONE PSUM operand per op, so
                            # m1 bounces through SBUF first (scalar engine),
                            # then even/odd chains run on vector/gpsimd.
                            add_ = mybir.AluOpType.add
                            sub_ = mybir.AluOpType.subtract
                            sl = act[:, j, 2 * t0 * W : 2 * (t0 + tn) * W].rearrange(
                                "p (t e w) -> p t e w", e=2, w=W
                            )
                            # engine split: scalar bounces m1/m2 out of PSUM,
                            # gpsimd (no PSUM access) does the SBUF-only
                            # combines, vector does the two PSUM-reading adds
                            rc_ctr += 1
                            c1 = scpool.tile([128, 512], f32, tag="rcc", bufs=2)
                            c2 = scpool.tile([128, 512], f32, tag="rcd", bufs=2)
                            u = scpool.tile([128, 512], f32, tag="rcu", bufs=2)
                            v = scpool.tile([128, 512], f32, tag="rcv", bufs=2)
                            m_r = [
                                m[:, :N].rearrange("p (t w) -> p t w", w=W) for m in ms
                            ]
                            ur = u[:, :N].rearrange("p (t w) -> p t w", w=W)
                            vr = v[:, :N].rearrange("p (t w) -> p t w", w=W)
                            nc.scalar.copy(c1[:, :N], ms[1][:, :N])
                            nc.scalar.copy(c2[:, :N], ms[2][:, :N])
                            nc.gpsimd.tensor_tensor(u[:, :N], c1[:, :N], c2[:, :N], add_)
                            nc.vector.tensor_tensor(sl[:, :, 0, :], ur, m_r[0], add_)
                            nc.gpsimd.tensor_tensor(v[:, :N], c1[:, :N], c2[:, :N], sub_)
                            nc.vector.tensor_tensor(sl[:, :, 1, :], vr, m_r[3], sub_)
                            if batile is not None:
                                for e in range(2):
                                    nc.vector.tensor_scalar(
                                        sl[:, :, e, :], sl[:, :, e, :], 1.0,
                                        batile[:, j : j + 1],
                                        mybir.AluOpType.mult, mybir.AluOpType.add,
                                    )
                            t0 += tn
                    if l == 2 and b == 2:
                        # transpose folded wh [6, px] -> [px-chunk, 6] on PE
                        # (via a spare conv PSUM buf), park in SBUF til decode
                        tmp = p3pool.tile([128, 512], f32, tag="blk", name="foldT")
                        for c in range(_NCH[2]):
                            M = min(128, S - c * 128)
                            nc.tensor.transpose(
                                tmp[:M, c * 6 : c * 6 + 6],
                                fhs[:, c * 128 : c * 128 + M],
                                ident[:6, :6],
                            )
                        fhT = spool.tile([128, _NCH[2], 6], f32, tag="fhT")
                        nc.vector.tensor_copy(
                            fhT[:],
                            tmp[:, : _NCH[2] * 6].rearrange("p (k c) -> p k c", c=6),
                        )

                    # ---- heads + decode for this branch (act is about to be
                    # overwritten by the other branch's conv) ----
                    wht = w2t if b == 2 else w3t
                    nch = NO2D if b == 2 else NO3D
                    bbx = bbt.get((l, b))
                    for (k0, gn) in _groups(_NCH[l], g=2 if l == 2 else 16):
                        hp = hpool.tile([128, 16, NO3D * 3], f32, tag="hp", name="hp")
                        for gi in range(gn):
                            px0 = (k0 + gi) * 128
                            M = min(128, S - px0)
                            for q in range(Q):
                                nc.tensor.matmul(
                                    hp[:M, gi, : nch * 3],
                                    act[:, q, px0 : px0 + M],
                                    wht[:, q, :],
                                    start=(q == 0),
                                    stop=(q == Q - 1),
                                )
                        if bbx is not None:
                            nc.vector.tensor_tensor(
                                hp[:, :gn, : nch * 3], hp[:, :gn, : nch * 3],
                                bbx[:, None, :].to_broadcast([128, gn, nch * 3]), add,
                            )
                        for a in range(3):
                            sta = st[l][a]
                            cols = sta[:, k0 : k0 + gn, :]
                            if b == 2:
                                # h2: sigmoid all 8 channels
                                nc.scalar.activation(cols[:, :, 0:NO2D], hp[:, :gn, NO2D * a : NO2D * (a + 1)], sig)
                                if fhT is not None:
                                    # wh from the exact folded fp16 path
                                    nc.scalar.activation(
                                        cols[:, :, 2:4], fhT[:, k0 : k0 + gn, 2 * a : 2 * a + 2], sig
                                    )
                                # xy: sig*2s + (grid-0.5)*s  (SBUF-only
                                # ops ride gpsimd; vector is recombine-bound)
                                nc.vector.tensor_scalar_mul(cols[:, :, 0:2], cols[:, :, 0:2], 2.0 * stride)
                                nc.vector.tensor_tensor(cols[:, :, 0:2], cols[:, :, 0:2], gts[l][:, k0 : k0 + gn, :], add)
                                # wh: (2 sig)^2 A = sig^2 * 4A
                                nc.vector.tensor_tensor(cols[:, :, 2:4], cols[:, :, 2:4], cols[:, :, 2:4], mult)
                                nc.vector.tensor_tensor(
                                    cols[:, :, 2:4], cols[:, :, 2:4],
                                    ancht[:, l, a, :][:, None, :].to_broadcast([128, gn, 2]), mult,
                                )
                            else:
                                # h3 bins+orient raw copy
                                nc.vector.tensor_copy(cols[:, :, 8:14], hp[:, :gn, NO3D * a : NO3D * a + 6])
                                # orient norm^2 -> norm buffer
                                sqt = scpool.tile([128, 16, 4], f32, tag="sqt")
                                nc.vector.tensor_tensor(sqt[:, :gn, :], cols[:, :, 10:14], cols[:, :, 10:14], mult)
                                sq4 = sqt[:, :gn, :].rearrange("p g (j t) -> p g j t", t=2)
                                noff = _NBOFF[l * 3 + a]
                                nc.vector.tensor_tensor(
                                    nb[:, noff + k0 : noff + k0 + gn, :], sq4[:, :, :, 0], sq4[:, :, :, 1], add
                                )
                                # dims: sigmoid * 2 - 1
                                nc.scalar.activation(cols[:, :, 14:17], hp[:, :gn, NO3D * a + 6 : NO3D * a + 9], sig)
                                nc.vector.tensor_scalar(cols[:, :, 14:17], cols[:, :, 14:17], 2.0, -1.0, mult, add)
                        if l == 2 and b == 2:
                            # stores ride out per group: first wave's DMA
                            # drain hides under the second group's decode
                            _emit_stores(2, S, k0, k0 + gn)
                    if l == 2 and b == 3:
                        # b3 runs FIRST at level 2: normalize orientation now
                        # so the sqrt-table load and multiplies hide under
                        # b2's conv instead of sitting in the exposed tail
                        _emit_norm(2)

                # orientation normalize + output DMA for this level; emission
                # deferred so the 68B-granular store flood lands in a window
                # where the DMA queues are not feeding the PE's weight stream
                def _emit_tail(l=l, S=S):
                    _emit_norm(l)
                    _emit_stores(l, S)

                pending_tails.append(_emit_tail)

    nc.finalize()
    return nc


def _pack_inputs(inputs):
    """Host-side packing: Winograd-y V planes, U-transformed fp8 weights."""
    shared = {}
    percore = [dict() for _ in range(BS)]
    for l, (C, H, W, stride, _) in enumerate(LEVELS):
        Q = C // 128
        Hh = H // 2
        S = H * W
        f = np.asarray(inputs[f"f{l}"])
        W2 = _W2P[l]
        np_dt = ml_dtypes.float8_e4m3 if FP8B[l][0] else np.float16
        xpad = np.zeros((BS, C, H + 2, W2), np.float32)
        xpad[:, :, 1 : H + 1, 1 : W + 1] = f
        V = np.zeros((BS, C, 4, Hh, W2), np.float32)
        for p in range(4):
            for r in range(4):
                c = _BT[p, r]
                if c != 0.0:
                    V[:, :, p] += c * xpad[:, :, r : r + 2 * Hh : 2]
        Vq = V.astype(np_dt)
        Vp = np.ascontiguousarray(
            Vq.reshape(BS, Q, 128, 4, Hh, W2).transpose(0, 2, 1, 3, 4, 5)
        )
        for bcore in range(BS):
            percore[bcore][f"v{l}"] = Vp[bcore]
        if l == 2:
            # fp16 V copy + folded (w2b_wh . w2a) U-weights for the exact
            # wh path; fp16 has ~16x less quantization noise than e4m3.
            V16 = np.ascontiguousarray(
                V.astype(np.float16).reshape(BS, Q, 128, 4, Hh, W2).transpose(0, 2, 1, 3, 4, 5)
            )
            for bcore in range(BS):
                percore[bcore]["v2h"] = V16[bcore]
            w2a = np.asarray(inputs["w2a2"]).astype(np.float32)
            w2b = np.asarray(inputs["w2b2"])[:, :, 0, 0].astype(np.float32)
            whr = [a * NO2D + k for a in range(3) for k in (2, 3)]
            wf = np.einsum("oc,cihw->oihw", w2b[whr], w2a)  # [6, C, 3, 3]
            Uf = np.einsum("pk,oikx->oipx", _G, wf)  # [6, C, 4, 3]
            shared["wfold"] = np.ascontiguousarray(
                Uf.reshape(6, Q, 128, 4, 3).transpose(2, 1, 3, 4, 0)
                .reshape(128, Q, 12, 6).astype(np.float16)
            )
            b2a = np.asarray(inputs["b2a2"]).astype(np.float32)
            b2b = np.asarray(inputs["b2b2"]).astype(np.float32)
            if np.any(b2a != 0) or np.any(b2b != 0):
                foldb = b2b[whr] + w2b[whr] @ b2a
                shared["foldb"] = np.ascontiguousarray(
                    foldb.reshape(6, 1).astype(np.float32)
                )

        for bi, (b, wkey) in enumerate(((2, f"w2a{l}"), (3, f"w3a{l}"))):
            w = np.asarray(inputs[wkey]).astype(np.float32)  # [C, C, 3, 3]
            U = np.einsum("pk,oikx->oipx", _G, w)  # [C, C, 4, 3]
            if FP8B[l][bi]:
                U = U * np.float32(WSCALE)
            wdt = ml_dtypes.float8_e4m3 if FP8B[l][bi] else np.float16
            U8 = U.reshape(Q, 128, Q, 128, 4, 3)  # [j, o, q, i, p, kx]
            shared[f"wa{b}{l}"] = np.ascontiguousarray(
                U8.transpose(0, 3, 2, 4, 5, 1).reshape(Q, 128, Q, 12, 128).astype(wdt)
            )
        for b, wkey, nch in ((2, f"w2b{l}", NO2D * 3), (3, f"w3b{l}", NO3D * 3)):
            w = np.asarray(inputs[wkey])[:, :, 0, 0].astype(np.float32)  # [nch, C]
            # fold the 1/WSCALE act rescale into the head weights
            wsc = w * np.float32(1.0 / WSCALE if FP8B[l][0] else 1.0)
            shared[f"wb{b}{l}"] = np.ascontiguousarray(
                wsc.T.reshape(Q, 128, nch).transpose(1, 0, 2).astype(np.float16)
            )

        px = np.arange(_NCH[l] * 128, dtype=np.float32)
        gx = np.where(px < S, px % W, 0.0).astype(np.float32)
        gy = np.where(px < S, px // W, 0.0).astype(np.float32)
        g = np.stack([(gx - 0.5) * stride, (gy - 0.5) * stride], -1)
        shared[f"grid{l}"] = np.ascontiguousarray(
            g.reshape(_NCH[l], 128, 2).transpose(1, 0, 2)
        )

    shared["anch"] = np.ascontiguousarray(
        np.broadcast_to(4.0 * ANCHORS[None], (128, 3, 3, 2)).astype(np.float32)
    )

    has_ba = []
    has_bb = []
    for l, (C, _, _, _, _) in enumerate(LEVELS):
        Q = C // 128
        for bi, (b, akey, bkey) in enumerate(((2, f"b2a{l}", f"b2b{l}"), (3, f"b3a{l}", f"b3b{l}"))):
            ba = np.asarray(inputs[akey])
            bb = np.asarray(inputs[bkey])
            nz_a = bool(np.any(ba != 0))
            nz_b = bool(np.any(bb != 0))
            has_ba.append(nz_a)
            has_bb.append(nz_b)
            if nz_a:
                # acts carry a WSCALE factor now; scale conv bias to match
                bsc = ba * np.float32(WSCALE if FP8B[l][bi] else 1.0)
                shared[f"ba{b}{l}"] = np.ascontiguousarray(
                    bsc.reshape(Q, 128).T.astype(np.float32)
                )
            if nz_b:
                shared[f"bb{b}{l}"] = np.ascontiguousarray(
                    np.broadcast_to(bb[None], (128, bb.shape[0])).astype(np.float32)
                )

    in_maps = []
    for bcore in range(BS):
        m = dict(shared)
        m.update(percore[bcore])
        in_maps.append(m)
    return in_maps, (tuple(has_ba), tuple(has_bb))


def _get_program(bias_flags):
    if bias_flags not in _PROGRAM_CACHE:
        _PROGRAM_CACHE[bias_flags] = _build_program(bias_flags)
    return _PROGRAM_CACHE[bias_flags]


def _run(inputs, trace=False):
    from concourse.bass_utils import run_bass_kernel_spmd

    in_maps, bias_flags = _pack_inputs(inputs)
    nc = _get_program(bias_flags)
    res = run_bass_kernel_spmd(
        nc, in_maps, core_ids=list(range(NCORES)), trace=trace
    )
    outp = np.stack([res.results[i]["out"] for i in range(NCORES)]).astype(np.float32)
    return outp, res


def kernel(**inputs) -> np.ndarray:
    outp, _ = _run(inputs, trace=False)
    return outp
